# revision 7
# baseline (speedup 1.0000x reference)
"""Multi-head self-attention (b=4, n=2048, f=1024, h=16) on 8 trn2 NeuronCores.

Sharding: core c -> batch c//2, head-half c%2 (8 heads of 64 dims each).
Each core computes its 8 heads' attention and a partial output projection
(attn_slice @ Wo_rows); host sums the two partials per batch and adds bo.

Device dataflow per core (all matmul operands bf16, PSUM fp32):
  qT/kT  = (x@Wq+bq)^T, (x@Wk+bk)^T  laid out [feat, tok]    (W stationary)
  v      = x@Wv+bv                   laid out [tok, feat]    (xT stationary)
  S1     = [v | 1]            per-head stationaries [tok, 65]
  S0     = e^{-m} * [v | 1]
  logitsT[j, i] = k_j . q_i   (keys on partitions, 2 heads row-packed in PE)
  Etil   = exp(logitsT/32 + m_j)     (ACT bias folds the additive mask for
                                      m_i=1 queries multiplicatively)
  A1/D1  = S1^T @ Etil  (masked numerator + denominator, ones-column trick)
  A0/D0  = S0^T @ Etil  (unmasked variant; e^{-m_j} undoes the bias)
  out_i  = m_i ? A1/D1 : A0/D0   (per-column select via host mask rows)
"""

import sys

sys.path.insert(0, "/opt/trn_rl_repo")

import numpy as np
import ml_dtypes

import concourse.bass as bass
import concourse.bacc as bacc
import concourse.mybir as mybir
import concourse.tile as tile
from concourse import bass_utils

BF16 = mybir.dt.bfloat16
F32 = mybir.dt.float32
NPBF16 = ml_dtypes.bfloat16

B, N, F, H, HD = 4, 2048, 1024, 16, 64
FH = 512          # features per core (8 heads)
NC_ = 8           # cores
NTOKC = N // 128  # 16 token chunks
NIBLK = N // 512  # 4 query blocks
NJ = N // 128     # 16 key chunks
NPAIR = 4         # head pairs per core
EXPFN = mybir.ActivationFunctionType.Exp


def _emit(nc, tc, d):
    """Emit the whole per-core program under TileContext tc.

    d: dict of dram tensor APs by name.
    """
    consts = tc.alloc_tile_pool(name="consts", bufs=1)
    persist = tc.alloc_tile_pool(name="persist", bufs=1)

    # ---- constant loads -------------------------------------------------
    mjb = consts.tile([128, NJ], F32)          # exp bias columns (m per key chunk)
    nc.sync.dma_start(out=mjb, in_=d["mjb"])
    emn = consts.tile([128, NTOKC], F32)       # e^{-m} per token chunk
    nc.sync.dma_start(out=emn, in_=d["emn"])
    mr4 = consts.tile([4, N], F32)             # rows [m, 1-m, m, 1-m]
    nc.sync.dma_start(out=mr4, in_=d["mr4"])
    bqk = consts.tile([128, 8], F32)           # bq chunks (0-3), bk chunks (4-7)
    nc.sync.dma_start(out=bqk, in_=d["bqk"])
    bvb = consts.tile([128, FH], F32)          # bv broadcast over partitions
    nc.sync.dma_start(out=bvb, in_=d["bvb"])
    wo_sb = consts.tile([128, 4 * 1024], BF16)  # Wo chunks, rows fc*128.. -> cols fc*1024..
    for fc in range(4):
        nc.sync.dma_start(
            out=wo_sb[:, fc * 1024:(fc + 1) * 1024],
            in_=d["wo"][fc * 128:(fc + 1) * 128, :],
        )

    # ---- persistent activations ----------------------------------------
    qT_sb = persist.tile([128, 4 * N], BF16)   # [feat, tok], chunk fc at cols fc*N
    kT_sb = persist.tile([128, 4 * N], BF16)
    s1_sb = persist.tile([128, NJ * 8 * 65], BF16)  # per (jc, head): [v | 1]
    s0_sb = persist.tile([128, NJ * 8 * 65], BF16)  # e^{-m} * [v | 1]
    attnT = persist.tile([128, 4 * N], BF16)   # normalized attn, [feat, tok]

    # ================= phase 1: projections ==============================
    with tc.tile_pool(name="p1sb", bufs=1) as p1sb, \
         tc.tile_pool(name="p1ps", bufs=2, space="PSUM") as p1ps, \
         tc.tile_pool(name="pvps", bufs=2, space="PSUM") as pvps:
        xT_sb = p1sb.tile([128, 8 * N], BF16)
        for fc in range(8):
            nc.sync.dma_start(
                out=xT_sb[:, fc * N:(fc + 1) * N],
                in_=d["xT"][fc * 128:(fc + 1) * 128, :],
            )
        w_sb = {}
        for wname in ("wq", "wk", "wv"):
            t = p1sb.tile([128, 8 * FH], BF16, tag=wname)
            for fc in range(8):
                nc.sync.dma_start(
                    out=t[:, fc * FH:(fc + 1) * FH],
                    in_=d[wname][fc * 128:(fc + 1) * 128, :],
                )
            w_sb[wname] = t

        # kT and qT: out[fh_chunk 128, tok] = W_chunk^T @ xT
        for wname, dst, boff in (("wk", kT_sb, 4), ("wq", qT_sb, 0)):
            for fhc in range(4):
                for half in range(2):
                    pk = p1ps.tile([128, 1024], F32, tag="projp", name="pk")
                    for fc in range(8):
                        lhsT = w_sb[wname][:, fc * FH + fhc * 128: fc * FH + (fhc + 1) * 128]
                        for nn in range(2):
                            off = half * 1024 + nn * 512
                            nc.tensor.matmul(
                                pk[:, nn * 512:(nn + 1) * 512],
                                lhsT,
                                xT_sb[:, fc * N + off: fc * N + off + 512],
                                start=(fc == 0),
                                stop=(fc == 7),
                            )
                    # copy psum -> sbuf with per-partition bias add
                    nc.vector.tensor_scalar_add(
                        out=dst[:, fhc * N + half * 1024: fhc * N + half * 1024 + 1024],
                        in0=pk[:],
                        scalar1=bqk[:, boff + fhc: boff + fhc + 1],
                    )

        # v: out[tok_chunk 128, fh 512] = xT_chunk^T @ Wv ; then build S1/S0
        for tokc in range(NTOKC):
            pv = pvps.tile([128, FH], F32, tag="vp")
            for fc in range(8):
                nc.tensor.matmul(
                    pv[:],
                    xT_sb[:, fc * N + tokc * 128: fc * N + (tokc + 1) * 128],
                    w_sb["wv"][:, fc * FH:(fc + 1) * FH],
                    start=(fc == 0),
                    stop=(fc == 7),
                )
            base = tokc * 8 * 65
            s1_v = s1_sb[:, base:base + 8 * 65].rearrange("p (h c) -> p h c", h=8)
            s0_v = s0_sb[:, base:base + 8 * 65].rearrange("p (h c) -> p h c", h=8)
            pv_v = pv[:].rearrange("p (h c) -> p h c", h=8)
            bv_v = bvb[:].rearrange("p (h c) -> p h c", h=8)
            # S1 = v + bv (head-strided dest, ones col at c=64)
            nc.vector.tensor_add(out=s1_v[:, :, 0:64], in0=pv_v, in1=bv_v)
            nc.vector.memset(s1_v[:, :, 64:65], 1.0)
            # S0 = e^{-m} * S1
            nc.vector.tensor_scalar_mul(
                out=s0_v[:, :, 0:64],
                in0=s1_v[:, :, 0:64],
                scalar1=emn[:, tokc:tokc + 1],
            )
            emn_b = bass.AP(
                tensor=emn.tensor,
                offset=emn[:, tokc:tokc + 1].offset,
                ap=[emn[:, tokc:tokc + 1].ap[0], [0, 8], [1, 1]],
            )
            nc.vector.tensor_copy(out=s0_v[:, :, 64:65], in_=emn_b)

    # ================= phase 2: attention ================================
    with tc.tile_pool(name="pP", bufs=2, space="PSUM") as pP, \
         tc.tile_pool(name="pacc", bufs=1, space="PSUM") as pacc, \
         tc.tile_pool(name="sexp", bufs=3) as sexp, \
         tc.tile_pool(name="episb", bufs=2) as episb, \
         tc.tile_pool(name="epidr", bufs=2, space="DRAM") as epidr:
        for iblk in range(NIBLK):
            for pair in range(NPAIR):
                accs = [
                    pacc.tile([65, 512], F32, tag=f"acc{v}", name=f"acc{v}")
                    for v in range(4)
                ]
                ptil = [None, None]

                def qk(j):
                    P = pP.tile([128, 1024], F32, tag="logits")
                    for hl, tp in ((0, 0), (1, 64)):
                        nc.tensor.matmul(
                            P[:, hl * 512:(hl + 1) * 512],
                            kT_sb[tp:tp + 64, pair * N + j * 128: pair * N + (j + 1) * 128],
                            qT_sb[tp:tp + 64, pair * N + iblk * 512: pair * N + (iblk + 1) * 512],
                            start=True,
                            stop=True,
                            tile_position=(tp, 0),
                        )
                    return P

                # software-pipelined emission: QK(j+1) lands between exp(j) and AV(j)
                Pcur = qk(0)
                for j in range(NJ):
                    S = sexp.tile([128, 1024], BF16, tag="etil")
                    nc.scalar.activation(
                        out=S[:], in_=Pcur[:], func=EXPFN,
                        bias=mjb[:, j:j + 1], scale=1.0 / 32.0,
                    )
                    if j + 1 < NJ:
                        Pcur = qk(j + 1)
                    for hl in range(2):
                        hcore = 2 * pair + hl
                        soff = j * 8 * 65 + hcore * 65
                        rhs = S[:, hl * 512:(hl + 1) * 512]
                        nc.tensor.matmul(
                            accs[2 * hl][:], s1_sb[:, soff:soff + 65], rhs,
                            start=(j == 0), stop=(j == NJ - 1),
                        )
                        nc.tensor.matmul(
                            accs[2 * hl + 1][:], s0_sb[:, soff:soff + 65], rhs,
                            start=(j == 0), stop=(j == NJ - 1),
                        )

                # ---- epilogue: select + normalize -----------------------
                asb = []
                for v in range(4):
                    t = episb.tile([65, 512], F32, tag=f"asb{v}", name=f"asb{v}")
                    nc.vector.tensor_copy(out=t, in_=accs[v][:])
                    asb.append(t)
                stg = epidr.tile([4, 512], F32, tag="stg")
                for v in range(4):
                    nc.sync.dma_start(out=stg[v:v + 1, :], in_=asb[v][64:65, :])
                rin = episb.tile([4, 512], F32, tag="rin")
                nc.sync.dma_start(out=rin, in_=stg[:])
                nc.vector.reciprocal(out=rin, in_=rin)
                nc.vector.tensor_mul(
                    out=rin, in0=rin, in1=mr4[:, iblk * 512:(iblk + 1) * 512]
                )
                stg2 = epidr.tile([4, 512], F32, tag="stg2")
                nc.sync.dma_start(out=stg2, in_=rin)
                rb = []
                for v in range(4):
                    t = episb.tile([64, 512], F32, tag=f"rb{v}", name=f"rb{v}")
                    src = stg2[v:v + 1, :]
                    nc.sync.dma_start(
                        out=t,
                        in_=bass.AP(tensor=src.tensor, offset=src.offset,
                                    ap=[[0, 64]] + src.ap[1:]),
                    )
                    rb.append(t)
                for hl in range(2):
                    t1 = episb.tile([64, 512], F32, tag="ept1")
                    t2 = episb.tile([64, 512], F32, tag="ept2")
                    nc.vector.tensor_mul(out=t1, in0=asb[2 * hl][0:64, :], in1=rb[2 * hl])
                    nc.vector.tensor_mul(out=t2, in0=asb[2 * hl + 1][0:64, :], in1=rb[2 * hl + 1])
                    dstc = pair * N + iblk * 512
                    if hl == 0:
                        nc.vector.tensor_add(
                            out=attnT[0:64, dstc:dstc + 512], in0=t1, in1=t2
                        )
                    else:
                        t3 = episb.tile([64, 512], BF16, tag="ept3")
                        nc.vector.tensor_add(out=t3, in0=t1, in1=t2)
                        nc.sync.dma_start(
                            out=attnT[64:128, dstc:dstc + 512], in_=t3
                        )

    # ================= phase 3: output projection ========================
    with tc.tile_pool(name="pO", bufs=2, space="PSUM") as pO, \
         tc.tile_pool(name="osb", bufs=2) as osb:
        for tokc in range(NTOKC):
            po = pO.tile([128, 1024], F32, tag="op")
            for fc in range(4):
                lhsT = attnT[:, fc * N + tokc * 128: fc * N + (tokc + 1) * 128]
                for nn in range(2):
                    nc.tensor.matmul(
                        po[:, nn * 512:(nn + 1) * 512],
                        lhsT,
                        wo_sb[:, fc * 1024 + nn * 512: fc * 1024 + (nn + 1) * 512],
                        start=(fc == 0),
                        stop=(fc == 3),
                    )
            ot = osb.tile([128, 1024], F32, tag="ot")
            nc.vector.tensor_copy(out=ot, in_=po[:])
            nc.sync.dma_start(
                out=d["y"][tokc * 128:(tokc + 1) * 128, :], in_=ot
            )

    persist.release()
    consts.release()


_CACHE = {}


def build_program():
    if "nc" in _CACHE:
        return _CACHE["nc"]
    nc = bacc.Bacc("TRN2", target_bir_lowering=False, debug=False)
    d = {}
    d["xT"] = nc.dram_tensor("xT", (F, N), BF16, kind="ExternalInput").ap()
    d["wq"] = nc.dram_tensor("wq", (F, FH), BF16, kind="ExternalInput").ap()
    d["wk"] = nc.dram_tensor("wk", (F, FH), BF16, kind="ExternalInput").ap()
    d["wv"] = nc.dram_tensor("wv", (F, FH), BF16, kind="ExternalInput").ap()
    d["wo"] = nc.dram_tensor("wo", (FH, F), BF16, kind="ExternalInput").ap()
    d["bqk"] = nc.dram_tensor("bqk", (128, 8), F32, kind="ExternalInput").ap()
    d["bvb"] = nc.dram_tensor("bvb", (128, FH), F32, kind="ExternalInput").ap()
    d["mjb"] = nc.dram_tensor("mjb", (128, NJ), F32, kind="ExternalInput").ap()
    d["emn"] = nc.dram_tensor("emn", (128, NTOKC), F32, kind="ExternalInput").ap()
    d["mr4"] = nc.dram_tensor("mr4", (4, N), F32, kind="ExternalInput").ap()
    d["y"] = nc.dram_tensor("y", (N, F), F32, kind="ExternalOutput").ap()
    with tile.TileContext(nc) as tc:
        _emit(nc, tc, d)
    nc.compile()
    _CACHE["nc"] = nc
    return nc


def make_in_maps(x, inputs_mask, Wq, bq, Wk, bk, Wv, bv, Wo, bo):
    """Host-side shard prep. All args np.float32/int32 full tensors."""
    in_maps = []
    m_all = inputs_mask.astype(np.float32)
    for c in range(NC_):
        b, hh = c // 2, c % 2
        cs = slice(hh * FH, (hh + 1) * FH)
        m = m_all[b]
        im = {
            "xT": np.ascontiguousarray(x[b].T).astype(NPBF16),
            "wq": Wq[:, cs].astype(NPBF16),
            "wk": Wk[:, cs].astype(NPBF16),
            "wv": Wv[:, cs].astype(NPBF16),
            "wo": np.ascontiguousarray(Wo[cs, :]).astype(NPBF16),
            "bqk": np.stack(
                [bq[cs].reshape(4, 128), bk[cs].reshape(4, 128)], axis=0
            ).reshape(8, 128).T.astype(np.float32).copy(),
            "bvb": np.broadcast_to(bv[cs], (128, FH)).astype(np.float32).copy(),
            "mjb": m.reshape(NJ, 128).T.astype(np.float32).copy(),
            "emn": np.exp(-m).reshape(NTOKC, 128).T.astype(np.float32).copy(),
            "mr4": np.stack([m, 1.0 - m, m, 1.0 - m]).astype(np.float32).copy(),
        }
        in_maps.append(im)
    return in_maps


def kernel(x, inputs_mask, Wq, bq, Wk, bk, Wv, bv, Wo, bo):
    x = np.asarray(x, dtype=np.float32)
    inputs_mask = np.asarray(inputs_mask)
    Wq, bq = np.asarray(Wq, np.float32), np.asarray(bq, np.float32)
    Wk, bk = np.asarray(Wk, np.float32), np.asarray(bk, np.float32)
    Wv, bv = np.asarray(Wv, np.float32), np.asarray(bv, np.float32)
    Wo, bo = np.asarray(Wo, np.float32), np.asarray(bo, np.float32)

    nc = build_program()
    in_maps = make_in_maps(x, inputs_mask, Wq, bq, Wk, bk, Wv, bv, Wo, bo)
    res = bass_utils.run_bass_kernel_spmd(nc, in_maps, core_ids=list(range(NC_)))
    out = np.empty((B, N, F), dtype=np.float32)
    for b in range(B):
        out[b] = res.results[2 * b]["y"] + res.results[2 * b + 1]["y"] + bo
    return out


# revision 12
# speedup vs baseline: 1.0494x; 1.0494x over previous
"""Multi-head self-attention (b=4, n=2048, f=1024, h=16) on 8 trn2 NeuronCores.

Sharding: core c -> batch c//2, head-half c%2 (8 heads of 64 dims each).
Each core computes its 8 heads' attention and a partial output projection
(attn_slice @ Wo_rows); host sums the two partials per batch and adds bo.

Device dataflow per core (all matmul operands bf16, PSUM fp32):
  qT/kT  = (x@Wq+bq)^T, (x@Wk+bk)^T  laid out [feat, tok]    (W stationary)
  v      = x@Wv+bv                   laid out [tok, feat]    (xT stationary)
  S1     = [v | 1]            per-head stationaries [tok, 65]
  S0     = e^{-m} * [v | 1]
  logitsT[j, i] = k_j . q_i   (keys on partitions, 2 heads row-packed in PE)
  Etil   = exp(logitsT/32 + m_j)     (ACT bias folds the additive mask for
                                      m_i=1 queries multiplicatively)
  A1/D1  = S1^T @ Etil  (masked numerator + denominator, ones-column trick)
  A0/D0  = S0^T @ Etil  (unmasked variant; e^{-m_j} undoes the bias)
  out_i  = m_i ? A1/D1 : A0/D0   (per-column select via host mask rows)
"""

import sys

sys.path.insert(0, "/opt/trn_rl_repo")

import numpy as np
import ml_dtypes

import concourse.bass as bass
import concourse.bacc as bacc
import concourse.mybir as mybir
import concourse.tile as tile
from concourse import bass_utils

BF16 = mybir.dt.bfloat16
F32 = mybir.dt.float32
NPBF16 = ml_dtypes.bfloat16

B, N, F, H, HD = 4, 2048, 1024, 16, 64
FH = 512          # features per core (8 heads)
NC_ = 8           # cores
NTOKC = N // 128  # 16 token chunks
NIBLK = N // 512  # 4 query blocks
NJ = N // 128     # 16 key chunks
NPAIR = 4         # head pairs per core
EXPFN = mybir.ActivationFunctionType.Exp


def _emit(nc, tc, d):
    """Emit the whole per-core program under TileContext tc.

    d: dict of dram tensor APs by name.
    """
    consts = tc.alloc_tile_pool(name="consts", bufs=1)
    persist = tc.alloc_tile_pool(name="persist", bufs=1)

    # ---- persistent activations ----------------------------------------
    qT_sb = persist.tile([128, 4 * N], BF16)   # [feat, tok], chunk fc at cols fc*N
    kT_sb = persist.tile([128, 4 * N], BF16)
    s1_sb = persist.tile([128, NJ * 8 * 65], BF16)  # per (jc, head): [v | 1]
    s0_sb = persist.tile([128, NJ * 8 * 65], BF16)  # e^{-m} * [v | 1]
    attnT = persist.tile([128, 4 * N], BF16)   # normalized attn, [feat, tok]

    # ================= phase 1: projections ==============================
    with tc.tile_pool(name="p1sb", bufs=1) as p1sb, \
         tc.tile_pool(name="pkt", bufs=1, space="PSUM") as pkt:
        # xT + Wk loads first (kT matmuls consume them chunk by chunk)
        xT_sb = p1sb.tile([128, 8 * N], BF16)
        wk_sb = p1sb.tile([128, 8 * FH], BF16)
        bqk = consts.tile([128, 8], F32)       # bq chunks (0-3), bk chunks (4-7)
        nc.sync.dma_start(out=bqk, in_=d["bqk"])
        for fc in range(8):
            nc.sync.dma_start(
                out=wk_sb[:, fc * FH:(fc + 1) * FH],
                in_=d["wk"][fc * 128:(fc + 1) * 128, :],
            )
            nc.sync.dma_start(
                out=xT_sb[:, fc * N:(fc + 1) * N],
                in_=d["xT"][fc * 128:(fc + 1) * 128, :],
            )

        # kT: fc-outer over 4 concurrent psum tiles (full PSUM) so the first
        # matmuls start as soon as chunk 0 of xT/Wk lands.
        for grp in range(2):
            pks = [
                pkt.tile([128, 1024], F32, tag=f"pp{t}", name=f"pk{t}")
                for t in range(4)
            ]
            for fc in range(8):
                for t in range(4):
                    fhc, half = grp * 2 + t // 2, t % 2
                    lhsT = wk_sb[:, fc * FH + fhc * 128: fc * FH + (fhc + 1) * 128]
                    for nn in range(2):
                        off = half * 1024 + nn * 512
                        nc.tensor.matmul(
                            pks[t][:, nn * 512:(nn + 1) * 512],
                            lhsT,
                            xT_sb[:, fc * N + off: fc * N + off + 512],
                            start=(fc == 0),
                            stop=(fc == 7),
                        )
            for t in range(4):
                fhc, half = grp * 2 + t // 2, t % 2
                nc.vector.tensor_scalar_add(
                    out=kT_sb[:, fhc * N + half * 1024: fhc * N + half * 1024 + 1024],
                    in0=pks[t][:],
                    scalar1=bqk[:, 4 + fhc: 5 + fhc],
                )

        # remaining loads (emitted after kT matmuls so they queue behind)
        mjb = consts.tile([128, NJ], F32)      # exp bias columns (m per key chunk)
        nc.sync.dma_start(out=mjb, in_=d["mjb"])
        emn = consts.tile([128, NTOKC], F32)   # e^{-m} per token chunk
        nc.sync.dma_start(out=emn, in_=d["emn"])
        mr4 = consts.tile([4, N], F32)         # rows [m, 1-m, m, 1-m]
        nc.sync.dma_start(out=mr4, in_=d["mr4"])
        bvb = consts.tile([128, FH], F32)      # bv broadcast over partitions
        nc.sync.dma_start(out=bvb, in_=d["bvb"])
        wo_sb = consts.tile([128, 4 * 1024], BF16)
        for fc in range(4):
            nc.sync.dma_start(
                out=wo_sb[:, fc * 1024:(fc + 1) * 1024],
                in_=d["wo"][fc * 128:(fc + 1) * 128, :],
            )
        w_sb = {}
        for wname in ("wq", "wv"):
            t = p1sb.tile([128, 8 * FH], BF16, tag=wname)
            for fc in range(8):
                nc.sync.dma_start(
                    out=t[:, fc * FH:(fc + 1) * FH],
                    in_=d[wname][fc * 128:(fc + 1) * 128, :],
                )
            w_sb[wname] = t

        # qT (xT is resident by now; fc-inner keeps PSUM small)
        for fhc in range(4):
            for half in range(2):
                pk = pkt.tile(
                    [128, 1024], F32, tag=f"pp{(fhc * 2 + half) % 2}", name="pkq"
                )
                for fc in range(8):
                    lhsT = w_sb["wq"][:, fc * FH + fhc * 128: fc * FH + (fhc + 1) * 128]
                    for nn in range(2):
                        off = half * 1024 + nn * 512
                        nc.tensor.matmul(
                            pk[:, nn * 512:(nn + 1) * 512],
                            lhsT,
                            xT_sb[:, fc * N + off: fc * N + off + 512],
                            start=(fc == 0),
                            stop=(fc == 7),
                        )
                nc.vector.tensor_scalar_add(
                    out=qT_sb[:, fhc * N + half * 1024: fhc * N + half * 1024 + 1024],
                    in0=pk[:],
                    scalar1=bqk[:, fhc: fhc + 1],
                )

        # v: out[tok_chunk 128, fh 512] = xT_chunk^T @ Wv ; then build S1/S0
        for tokc in range(NTOKC):
            pv = pkt.tile([128, FH], F32, tag=f"pp{2 + tokc % 2}", name="pv")
            for fc in range(8):
                nc.tensor.matmul(
                    pv[:],
                    xT_sb[:, fc * N + tokc * 128: fc * N + (tokc + 1) * 128],
                    w_sb["wv"][:, fc * FH:(fc + 1) * FH],
                    start=(fc == 0),
                    stop=(fc == 7),
                )
            base = tokc * 8 * 65
            s1_v = s1_sb[:, base:base + 8 * 65].rearrange("p (h c) -> p h c", h=8)
            s0_v = s0_sb[:, base:base + 8 * 65].rearrange("p (h c) -> p h c", h=8)
            pv_v = pv[:].rearrange("p (h c) -> p h c", h=8)
            bv_v = bvb[:].rearrange("p (h c) -> p h c", h=8)
            # S1 = v + bv (head-strided dest, ones col at c=64)
            nc.vector.tensor_add(out=s1_v[:, :, 0:64], in0=pv_v, in1=bv_v)
            nc.vector.memset(s1_v[:, :, 64:65], 1.0)
            # S0 = e^{-m} * S1
            nc.vector.tensor_scalar_mul(
                out=s0_v[:, :, 0:64],
                in0=s1_v[:, :, 0:64],
                scalar1=emn[:, tokc:tokc + 1],
            )
            emn_b = bass.AP(
                tensor=emn.tensor,
                offset=emn[:, tokc:tokc + 1].offset,
                ap=[emn[:, tokc:tokc + 1].ap[0], [0, 8], [1, 1]],
            )
            nc.vector.tensor_copy(out=s0_v[:, :, 64:65], in_=emn_b)

    # ================= phase 2: attention ================================
    with tc.tile_pool(name="pP", bufs=2, space="PSUM") as pP, \
         tc.tile_pool(name="pacc", bufs=1, space="PSUM") as pacc, \
         tc.tile_pool(name="sexp", bufs=3) as sexp, \
         tc.tile_pool(name="episb", bufs=2) as episb, \
         tc.tile_pool(name="epidr", bufs=2, space="DRAM") as epidr:
        for iblk in range(NIBLK):
            for pair in range(NPAIR):
                accs = [
                    pacc.tile([65, 512], F32, tag=f"acc{v}", name=f"acc{v}")
                    for v in range(4)
                ]
                ptil = [None, None]

                def qk(j):
                    P = pP.tile([128, 1024], F32, tag="logits")
                    for hl, tp in ((0, 0), (1, 64)):
                        nc.tensor.matmul(
                            P[:, hl * 512:(hl + 1) * 512],
                            kT_sb[tp:tp + 64, pair * N + j * 128: pair * N + (j + 1) * 128],
                            qT_sb[tp:tp + 64, pair * N + iblk * 512: pair * N + (iblk + 1) * 512],
                            start=True,
                            stop=True,
                            tile_position=(tp, 0),
                        )
                    return P

                # software-pipelined emission: QK(j+1) lands between exp(j) and AV(j)
                Pcur = qk(0)
                for j in range(NJ):
                    S = sexp.tile([128, 1024], BF16, tag="etil")
                    nc.scalar.activation(
                        out=S[:], in_=Pcur[:], func=EXPFN,
                        bias=mjb[:, j:j + 1], scale=1.0 / 32.0,
                    )
                    if j + 1 < NJ:
                        Pcur = qk(j + 1)
                    for hl in range(2):
                        hcore = 2 * pair + hl
                        soff = j * 8 * 65 + hcore * 65
                        rhs = S[:, hl * 512:(hl + 1) * 512]
                        nc.tensor.matmul(
                            accs[2 * hl][:], s1_sb[:, soff:soff + 65], rhs,
                            start=(j == 0), stop=(j == NJ - 1),
                        )
                        nc.tensor.matmul(
                            accs[2 * hl + 1][:], s0_sb[:, soff:soff + 65], rhs,
                            start=(j == 0), stop=(j == NJ - 1),
                        )

                # ---- epilogue: select + normalize -----------------------
                asb = []
                for v in range(4):
                    t = episb.tile([65, 512], F32, tag=f"asb{v}", name=f"asb{v}")
                    nc.vector.tensor_copy(out=t, in_=accs[v][:])
                    asb.append(t)
                rin = episb.tile([4, 512], F32, tag="rin")
                for v in range(4):
                    nc.sync.dma_start(out=rin[v:v + 1, :], in_=asb[v][64:65, :])
                rsc = episb.tile([4, 512], F32, tag="rsc")
                nc.vector.reciprocal_approx_fast(out=rsc, in_=rin)
                nc.vector.tensor_mul(
                    out=rsc, in0=rsc, in1=mr4[:, iblk * 512:(iblk + 1) * 512]
                )
                stg2 = epidr.tile([4, 512], F32, tag="stg2")
                nc.sync.dma_start(out=stg2, in_=rsc)
                rball = episb.tile([64, 4 * 512], F32, tag="rball")
                nc.sync.dma_start(
                    out=rball,
                    in_=bass.AP(tensor=stg2.tensor, offset=stg2.offset,
                                ap=[[0, 64], [512, 4], [1, 512]]),
                )
                rb = [rball[:, v * 512:(v + 1) * 512] for v in range(4)]
                for hl in range(2):
                    t1 = episb.tile([64, 512], F32, tag="ept1")
                    t2 = episb.tile([64, 512], F32, tag="ept2")
                    nc.vector.tensor_mul(out=t1, in0=asb[2 * hl][0:64, :], in1=rb[2 * hl])
                    nc.vector.tensor_mul(out=t2, in0=asb[2 * hl + 1][0:64, :], in1=rb[2 * hl + 1])
                    dstc = pair * N + iblk * 512
                    if hl == 0:
                        nc.vector.tensor_add(
                            out=attnT[0:64, dstc:dstc + 512], in0=t1, in1=t2
                        )
                    else:
                        t3 = episb.tile([64, 512], BF16, tag="ept3")
                        nc.vector.tensor_add(out=t3, in0=t1, in1=t2)
                        nc.sync.dma_start(
                            out=attnT[64:128, dstc:dstc + 512], in_=t3
                        )

    # ================= phase 3: output projection ========================
    with tc.tile_pool(name="pO", bufs=2, space="PSUM") as pO, \
         tc.tile_pool(name="osb", bufs=2) as osb:
        for tokc in range(NTOKC):
            po = pO.tile([128, 1024], F32, tag="op")
            for fc in range(4):
                lhsT = attnT[:, fc * N + tokc * 128: fc * N + (tokc + 1) * 128]
                for nn in range(2):
                    nc.tensor.matmul(
                        po[:, nn * 512:(nn + 1) * 512],
                        lhsT,
                        wo_sb[:, fc * 1024 + nn * 512: fc * 1024 + (nn + 1) * 512],
                        start=(fc == 0),
                        stop=(fc == 3),
                    )
            ot = osb.tile([128, 1024], F32, tag="ot")
            # ScalarE copy: keeps the O-proj drain off the DVE queue, which is
            # still flushing the last attention epilogue.
            nc.scalar.activation(
                out=ot, in_=po[:], func=mybir.ActivationFunctionType.Copy
            )
            nc.sync.dma_start(
                out=d["y"][tokc * 128:(tokc + 1) * 128, :], in_=ot
            )

    persist.release()
    consts.release()


_CACHE = {}


def build_program():
    if "nc" in _CACHE:
        return _CACHE["nc"]
    nc = bacc.Bacc("TRN2", target_bir_lowering=False, debug=False)
    d = {}
    d["xT"] = nc.dram_tensor("xT", (F, N), BF16, kind="ExternalInput").ap()
    d["wq"] = nc.dram_tensor("wq", (F, FH), BF16, kind="ExternalInput").ap()
    d["wk"] = nc.dram_tensor("wk", (F, FH), BF16, kind="ExternalInput").ap()
    d["wv"] = nc.dram_tensor("wv", (F, FH), BF16, kind="ExternalInput").ap()
    d["wo"] = nc.dram_tensor("wo", (FH, F), BF16, kind="ExternalInput").ap()
    d["bqk"] = nc.dram_tensor("bqk", (128, 8), F32, kind="ExternalInput").ap()
    d["bvb"] = nc.dram_tensor("bvb", (128, FH), F32, kind="ExternalInput").ap()
    d["mjb"] = nc.dram_tensor("mjb", (128, NJ), F32, kind="ExternalInput").ap()
    d["emn"] = nc.dram_tensor("emn", (128, NTOKC), F32, kind="ExternalInput").ap()
    d["mr4"] = nc.dram_tensor("mr4", (4, N), F32, kind="ExternalInput").ap()
    d["y"] = nc.dram_tensor("y", (N, F), F32, kind="ExternalOutput").ap()
    with tile.TileContext(nc) as tc:
        _emit(nc, tc, d)
    nc.compile()
    _CACHE["nc"] = nc
    return nc


def make_in_maps(x, inputs_mask, Wq, bq, Wk, bk, Wv, bv, Wo, bo):
    """Host-side shard prep. All args np.float32/int32 full tensors."""
    in_maps = []
    m_all = inputs_mask.astype(np.float32)
    for c in range(NC_):
        b, hh = c // 2, c % 2
        cs = slice(hh * FH, (hh + 1) * FH)
        m = m_all[b]
        im = {
            "xT": np.ascontiguousarray(x[b].T).astype(NPBF16),
            "wq": Wq[:, cs].astype(NPBF16),
            "wk": Wk[:, cs].astype(NPBF16),
            "wv": Wv[:, cs].astype(NPBF16),
            "wo": np.ascontiguousarray(Wo[cs, :]).astype(NPBF16),
            "bqk": np.stack(
                [bq[cs].reshape(4, 128), bk[cs].reshape(4, 128)], axis=0
            ).reshape(8, 128).T.astype(np.float32).copy(),
            "bvb": np.broadcast_to(bv[cs], (128, FH)).astype(np.float32).copy(),
            "mjb": m.reshape(NJ, 128).T.astype(np.float32).copy(),
            "emn": np.exp(-m).reshape(NTOKC, 128).T.astype(np.float32).copy(),
            "mr4": np.stack([m, 1.0 - m, m, 1.0 - m]).astype(np.float32).copy(),
        }
        in_maps.append(im)
    return in_maps


def kernel(x, inputs_mask, Wq, bq, Wk, bk, Wv, bv, Wo, bo):
    x = np.asarray(x, dtype=np.float32)
    inputs_mask = np.asarray(inputs_mask)
    Wq, bq = np.asarray(Wq, np.float32), np.asarray(bq, np.float32)
    Wk, bk = np.asarray(Wk, np.float32), np.asarray(bk, np.float32)
    Wv, bv = np.asarray(Wv, np.float32), np.asarray(bv, np.float32)
    Wo, bo = np.asarray(Wo, np.float32), np.asarray(bo, np.float32)

    nc = build_program()
    in_maps = make_in_maps(x, inputs_mask, Wq, bq, Wk, bk, Wv, bv, Wo, bo)
    res = bass_utils.run_bass_kernel_spmd(nc, in_maps, core_ids=list(range(NC_)))
    out = np.empty((B, N, F), dtype=np.float32)
    for b in range(B):
        out[b] = res.results[2 * b]["y"] + res.results[2 * b + 1]["y"] + bo
    return out


# revision 13
# speedup vs baseline: 1.0865x; 1.0354x over previous
"""Multi-head self-attention (b=4, n=2048, f=1024, h=16) on 8 trn2 NeuronCores.

Sharding: core c -> batch c//2, head-half c%2 (8 heads of 64 dims each).
Each core computes its 8 heads' attention and a partial output projection
(attn_slice @ Wo_rows); host sums the two partials per batch and adds bo.

Device dataflow per core (all matmul operands bf16, PSUM fp32):
  qT/kT  = (x@Wq+bq)^T, (x@Wk+bk)^T  laid out [feat, tok]    (W stationary)
  v      = x@Wv+bv                   laid out [tok, feat]    (xT stationary)
  S1     = [v | 1]            per-head stationaries [tok, 65]
  S0     = e^{-m} * [v | 1]
  logitsT[j, i] = k_j . q_i   (keys on partitions, 2 heads row-packed in PE)
  Etil   = exp(logitsT/32 + m_j)     (ACT bias folds the additive mask for
                                      m_i=1 queries multiplicatively)
  A1/D1  = S1^T @ Etil  (masked numerator + denominator, ones-column trick)
  A0/D0  = S0^T @ Etil  (unmasked variant; e^{-m_j} undoes the bias)
  out_i  = m_i ? A1/D1 : A0/D0   (per-column select via host mask rows)
"""

import sys

sys.path.insert(0, "/opt/trn_rl_repo")

import numpy as np
import ml_dtypes

import concourse.bass as bass
import concourse.bacc as bacc
import concourse.mybir as mybir
import concourse.tile as tile
from concourse import bass_utils

BF16 = mybir.dt.bfloat16
F32 = mybir.dt.float32
NPBF16 = ml_dtypes.bfloat16

B, N, F, H, HD = 4, 2048, 1024, 16, 64
FH = 512          # features per core (8 heads)
NC_ = 8           # cores
NTOKC = N // 128  # 16 token chunks
NIBLK = N // 512  # 4 query blocks
NJ = N // 128     # 16 key chunks
NPAIR = 4         # head pairs per core
EXPFN = mybir.ActivationFunctionType.Exp


def _emit(nc, tc, d):
    """Emit the whole per-core program under TileContext tc.

    d: dict of dram tensor APs by name.
    """
    consts = tc.alloc_tile_pool(name="consts", bufs=1)
    persist = tc.alloc_tile_pool(name="persist", bufs=1)

    # ---- persistent activations ----------------------------------------
    qT_sb = persist.tile([128, 4 * N], BF16)   # [feat, tok], chunk fc at cols fc*N
    kT_sb = persist.tile([128, 4 * N], BF16)
    s1_sb = persist.tile([128, NJ * 8 * 65], BF16)  # per (jc, head): [v | 1]
    s0_sb = persist.tile([128, NJ * 8 * 65], BF16)  # e^{-m} * [v | 1]
    attnT = persist.tile([128, 4 * N], BF16)   # normalized attn, [feat, tok]

    # ================= phase 1: projections ==============================
    with tc.tile_pool(name="p1sb", bufs=1) as p1sb, \
         tc.tile_pool(name="pkt", bufs=1, space="PSUM") as pkt:
        # xT + Wk loads first (kT matmuls consume them chunk by chunk)
        xT_sb = p1sb.tile([128, 8 * N], BF16)
        wk_sb = p1sb.tile([128, 8 * FH], BF16)
        bqk = consts.tile([128, 8], F32)       # bq chunks (0-3), bk chunks (4-7)
        nc.sync.dma_start(out=bqk, in_=d["bqk"])
        for fc in range(8):
            nc.sync.dma_start(
                out=wk_sb[:, fc * FH:(fc + 1) * FH],
                in_=d["wk"][fc * 128:(fc + 1) * 128, :],
            )
            nc.sync.dma_start(
                out=xT_sb[:, fc * N:(fc + 1) * N],
                in_=d["xT"][fc * 128:(fc + 1) * 128, :],
            )

        # kT: fc-outer over 4 concurrent psum tiles (full PSUM) so the first
        # matmuls start as soon as chunk 0 of xT/Wk lands.
        for grp in range(2):
            pks = [
                pkt.tile([128, 1024], F32, tag=f"pp{t}", name=f"pk{t}")
                for t in range(4)
            ]
            for fc in range(8):
                for t in range(4):
                    fhc, half = grp * 2 + t // 2, t % 2
                    lhsT = wk_sb[:, fc * FH + fhc * 128: fc * FH + (fhc + 1) * 128]
                    for nn in range(2):
                        off = half * 1024 + nn * 512
                        nc.tensor.matmul(
                            pks[t][:, nn * 512:(nn + 1) * 512],
                            lhsT,
                            xT_sb[:, fc * N + off: fc * N + off + 512],
                            start=(fc == 0),
                            stop=(fc == 7),
                        )
            for t in range(4):
                fhc, half = grp * 2 + t // 2, t % 2
                nc.vector.tensor_scalar_add(
                    out=kT_sb[:, fhc * N + half * 1024: fhc * N + half * 1024 + 1024],
                    in0=pks[t][:],
                    scalar1=bqk[:, 4 + fhc: 5 + fhc],
                )

        # remaining loads (emitted after kT matmuls so they queue behind)
        mjb = consts.tile([128, NJ], F32)      # exp bias columns (m per key chunk)
        nc.sync.dma_start(out=mjb, in_=d["mjb"])
        emn = consts.tile([128, NTOKC], F32)   # e^{-m} per token chunk
        nc.sync.dma_start(out=emn, in_=d["emn"])
        mr4 = consts.tile([4, N], F32)         # rows [m, 1-m, m, 1-m]
        nc.sync.dma_start(out=mr4, in_=d["mr4"])
        bvb = consts.tile([128, FH], F32)      # bv broadcast over partitions
        nc.sync.dma_start(out=bvb, in_=d["bvb"])
        wo_sb = consts.tile([128, 4 * 1024], BF16)
        for fc in range(4):
            nc.sync.dma_start(
                out=wo_sb[:, fc * 1024:(fc + 1) * 1024],
                in_=d["wo"][fc * 128:(fc + 1) * 128, :],
            )
        w_sb = {}
        for wname in ("wq", "wv"):
            t = p1sb.tile([128, 8 * FH], BF16, tag=wname)
            for fc in range(8):
                nc.sync.dma_start(
                    out=t[:, fc * FH:(fc + 1) * FH],
                    in_=d[wname][fc * 128:(fc + 1) * 128, :],
                )
            w_sb[wname] = t

        # qT (xT is resident by now; fc-inner keeps PSUM small)
        for fhc in range(4):
            for half in range(2):
                pk = pkt.tile(
                    [128, 1024], F32, tag=f"pp{(fhc * 2 + half) % 2}", name="pkq"
                )
                for fc in range(8):
                    lhsT = w_sb["wq"][:, fc * FH + fhc * 128: fc * FH + (fhc + 1) * 128]
                    for nn in range(2):
                        off = half * 1024 + nn * 512
                        nc.tensor.matmul(
                            pk[:, nn * 512:(nn + 1) * 512],
                            lhsT,
                            xT_sb[:, fc * N + off: fc * N + off + 512],
                            start=(fc == 0),
                            stop=(fc == 7),
                        )
                nc.vector.tensor_scalar_add(
                    out=qT_sb[:, fhc * N + half * 1024: fhc * N + half * 1024 + 1024],
                    in0=pk[:],
                    scalar1=bqk[:, fhc: fhc + 1],
                )

        # v: out[tok_chunk 128, fh 512] = xT_chunk^T @ Wv ; then build S1/S0
        for tokc in range(NTOKC):
            pv = pkt.tile([128, FH], F32, tag=f"pp{2 + tokc % 2}", name="pv")
            for fc in range(8):
                nc.tensor.matmul(
                    pv[:],
                    xT_sb[:, fc * N + tokc * 128: fc * N + (tokc + 1) * 128],
                    w_sb["wv"][:, fc * FH:(fc + 1) * FH],
                    start=(fc == 0),
                    stop=(fc == 7),
                )
            base = tokc * 8 * 65
            s1_v = s1_sb[:, base:base + 8 * 65].rearrange("p (h c) -> p h c", h=8)
            s0_v = s0_sb[:, base:base + 8 * 65].rearrange("p (h c) -> p h c", h=8)
            pv_v = pv[:].rearrange("p (h c) -> p h c", h=8)
            bv_v = bvb[:].rearrange("p (h c) -> p h c", h=8)
            # S1 = v + bv (head-strided dest, ones col at c=64)
            nc.vector.tensor_add(out=s1_v[:, :, 0:64], in0=pv_v, in1=bv_v)
            nc.vector.memset(s1_v[:, :, 64:65], 1.0)
            # S0 = e^{-m} * S1
            nc.vector.tensor_scalar_mul(
                out=s0_v[:, :, 0:64],
                in0=s1_v[:, :, 0:64],
                scalar1=emn[:, tokc:tokc + 1],
            )
            emn_b = bass.AP(
                tensor=emn.tensor,
                offset=emn[:, tokc:tokc + 1].offset,
                ap=[emn[:, tokc:tokc + 1].ap[0], [0, 8], [1, 1]],
            )
            nc.vector.tensor_copy(out=s0_v[:, :, 64:65], in_=emn_b)

    # ================= phase 2: attention ================================
    with tc.tile_pool(name="pP", bufs=2, space="PSUM") as pP, \
         tc.tile_pool(name="pacc", bufs=1, space="PSUM") as pacc, \
         tc.tile_pool(name="sexp", bufs=3) as sexp, \
         tc.tile_pool(name="episb", bufs=2) as episb, \
         tc.tile_pool(name="epidr", bufs=2, space="DRAM") as epidr:
        for iblk in range(NIBLK):
            for pair in range(NPAIR):
                accs = [
                    pacc.tile([65, 512], F32, tag=f"acc{v}", name=f"acc{v}")
                    for v in range(4)
                ]
                ptil = [None, None]

                def qk(j):
                    P = pP.tile([128, 1024], F32, tag="logits")
                    for hl, tp in ((0, 0), (1, 64)):
                        nc.tensor.matmul(
                            P[:, hl * 512:(hl + 1) * 512],
                            kT_sb[tp:tp + 64, pair * N + j * 128: pair * N + (j + 1) * 128],
                            qT_sb[tp:tp + 64, pair * N + iblk * 512: pair * N + (iblk + 1) * 512],
                            start=True,
                            stop=True,
                            tile_position=(tp, 0),
                        )
                    return P

                # software-pipelined emission: QK(j+1) lands between exp(j) and AV(j)
                Pcur = qk(0)
                for j in range(NJ):
                    S = sexp.tile([128, 1024], BF16, tag="etil")
                    nc.scalar.activation(
                        out=S[:], in_=Pcur[:], func=EXPFN,
                        bias=mjb[:, j:j + 1], scale=1.0 / 32.0,
                    )
                    if j + 1 < NJ:
                        Pcur = qk(j + 1)
                    for hl in range(2):
                        hcore = 2 * pair + hl
                        soff = j * 8 * 65 + hcore * 65
                        rhs = S[:, hl * 512:(hl + 1) * 512]
                        nc.tensor.matmul(
                            accs[2 * hl][:], s1_sb[:, soff:soff + 65], rhs,
                            start=(j == 0), stop=(j == NJ - 1),
                        )
                        nc.tensor.matmul(
                            accs[2 * hl + 1][:], s0_sb[:, soff:soff + 65], rhs,
                            start=(j == 0), stop=(j == NJ - 1),
                        )

                # ---- epilogue: select + normalize -----------------------
                asb = []
                for v in range(4):
                    t = episb.tile([65, 512], F32, tag=f"asb{v}", name=f"asb{v}")
                    nc.vector.tensor_copy(out=t, in_=accs[v][:])
                    asb.append(t)
                rin = episb.tile([4, 512], F32, tag="rin")
                for v in range(4):
                    nc.sync.dma_start(out=rin[v:v + 1, :], in_=asb[v][64:65, :])
                rsc = episb.tile([4, 512], F32, tag="rsc")
                nc.vector.reciprocal_approx_fast(out=rsc, in_=rin)
                nc.vector.tensor_mul(
                    out=rsc, in0=rsc, in1=mr4[:, iblk * 512:(iblk + 1) * 512]
                )
                stg2 = epidr.tile([4, 512], F32, tag="stg2")
                nc.sync.dma_start(out=stg2, in_=rsc)
                rball = episb.tile([64, 4 * 512], F32, tag="rball")
                nc.sync.dma_start(
                    out=rball,
                    in_=bass.AP(tensor=stg2.tensor, offset=stg2.offset,
                                ap=[[0, 64], [512, 4], [1, 512]]),
                )
                rb = [rball[:, v * 512:(v + 1) * 512] for v in range(4)]
                for hl in range(2):
                    t1 = episb.tile([64, 512], F32, tag="ept1")
                    t2 = episb.tile([64, 512], F32, tag="ept2")
                    nc.vector.tensor_mul(out=t1, in0=asb[2 * hl][0:64, :], in1=rb[2 * hl])
                    nc.vector.tensor_mul(out=t2, in0=asb[2 * hl + 1][0:64, :], in1=rb[2 * hl + 1])
                    dstc = pair * N + iblk * 512
                    if hl == 0:
                        nc.vector.tensor_add(
                            out=attnT[0:64, dstc:dstc + 512], in0=t1, in1=t2
                        )
                    else:
                        t3 = episb.tile([64, 512], BF16, tag="ept3")
                        nc.vector.tensor_add(out=t3, in0=t1, in1=t2)
                        nc.sync.dma_start(
                            out=attnT[64:128, dstc:dstc + 512], in_=t3
                        )

        # ===== phase 3: output projection (same pools — no scope barrier,
        # so O-proj matmuls start while the last epilogue drains) ==========
        with tc.tile_pool(name="osb", bufs=2) as osb:
            for tokc in range(NTOKC):
                po = pP.tile([128, 1024], F32, tag="logits", name="po")
                for fc in range(4):
                    lhsT = attnT[:, fc * N + tokc * 128: fc * N + (tokc + 1) * 128]
                    for nn in range(2):
                        nc.tensor.matmul(
                            po[:, nn * 512:(nn + 1) * 512],
                            lhsT,
                            wo_sb[:, fc * 1024 + nn * 512: fc * 1024 + (nn + 1) * 512],
                            start=(fc == 0),
                            stop=(fc == 3),
                        )
                ot = osb.tile([128, 1024], F32, tag="ot")
                # ScalarE copy: keeps the O-proj drain off the DVE queue,
                # which is still flushing the last attention epilogue.
                nc.scalar.activation(
                    out=ot, in_=po[:], func=mybir.ActivationFunctionType.Copy
                )
                nc.sync.dma_start(
                    out=d["y"][tokc * 128:(tokc + 1) * 128, :], in_=ot
                )

    persist.release()
    consts.release()


_CACHE = {}


def build_program():
    if "nc" in _CACHE:
        return _CACHE["nc"]
    nc = bacc.Bacc("TRN2", target_bir_lowering=False, debug=False)
    d = {}
    d["xT"] = nc.dram_tensor("xT", (F, N), BF16, kind="ExternalInput").ap()
    d["wq"] = nc.dram_tensor("wq", (F, FH), BF16, kind="ExternalInput").ap()
    d["wk"] = nc.dram_tensor("wk", (F, FH), BF16, kind="ExternalInput").ap()
    d["wv"] = nc.dram_tensor("wv", (F, FH), BF16, kind="ExternalInput").ap()
    d["wo"] = nc.dram_tensor("wo", (FH, F), BF16, kind="ExternalInput").ap()
    d["bqk"] = nc.dram_tensor("bqk", (128, 8), F32, kind="ExternalInput").ap()
    d["bvb"] = nc.dram_tensor("bvb", (128, FH), F32, kind="ExternalInput").ap()
    d["mjb"] = nc.dram_tensor("mjb", (128, NJ), F32, kind="ExternalInput").ap()
    d["emn"] = nc.dram_tensor("emn", (128, NTOKC), F32, kind="ExternalInput").ap()
    d["mr4"] = nc.dram_tensor("mr4", (4, N), F32, kind="ExternalInput").ap()
    d["y"] = nc.dram_tensor("y", (N, F), F32, kind="ExternalOutput").ap()
    with tile.TileContext(nc) as tc:
        _emit(nc, tc, d)
    nc.compile()
    _CACHE["nc"] = nc
    return nc


def make_in_maps(x, inputs_mask, Wq, bq, Wk, bk, Wv, bv, Wo, bo):
    """Host-side shard prep. All args np.float32/int32 full tensors."""
    in_maps = []
    m_all = inputs_mask.astype(np.float32)
    for c in range(NC_):
        b, hh = c // 2, c % 2
        cs = slice(hh * FH, (hh + 1) * FH)
        m = m_all[b]
        im = {
            "xT": np.ascontiguousarray(x[b].T).astype(NPBF16),
            "wq": Wq[:, cs].astype(NPBF16),
            "wk": Wk[:, cs].astype(NPBF16),
            "wv": Wv[:, cs].astype(NPBF16),
            "wo": np.ascontiguousarray(Wo[cs, :]).astype(NPBF16),
            "bqk": np.stack(
                [bq[cs].reshape(4, 128), bk[cs].reshape(4, 128)], axis=0
            ).reshape(8, 128).T.astype(np.float32).copy(),
            "bvb": np.broadcast_to(bv[cs], (128, FH)).astype(np.float32).copy(),
            "mjb": m.reshape(NJ, 128).T.astype(np.float32).copy(),
            "emn": np.exp(-m).reshape(NTOKC, 128).T.astype(np.float32).copy(),
            "mr4": np.stack([m, 1.0 - m, m, 1.0 - m]).astype(np.float32).copy(),
        }
        in_maps.append(im)
    return in_maps


def kernel(x, inputs_mask, Wq, bq, Wk, bk, Wv, bv, Wo, bo):
    x = np.asarray(x, dtype=np.float32)
    inputs_mask = np.asarray(inputs_mask)
    Wq, bq = np.asarray(Wq, np.float32), np.asarray(bq, np.float32)
    Wk, bk = np.asarray(Wk, np.float32), np.asarray(bk, np.float32)
    Wv, bv = np.asarray(Wv, np.float32), np.asarray(bv, np.float32)
    Wo, bo = np.asarray(Wo, np.float32), np.asarray(bo, np.float32)

    nc = build_program()
    in_maps = make_in_maps(x, inputs_mask, Wq, bq, Wk, bk, Wv, bv, Wo, bo)
    res = bass_utils.run_bass_kernel_spmd(nc, in_maps, core_ids=list(range(NC_)))
    out = np.empty((B, N, F), dtype=np.float32)
    for b in range(B):
        out[b] = res.results[2 * b]["y"] + res.results[2 * b + 1]["y"] + bo
    return out
